# revision 33
# baseline (speedup 1.0000x reference)
"""ALBertQALayer beam-search head on 8 Trainium2 NeuronCores.

Data-parallel: B=16 examples sharded 2-per-core; all params replicated.
Math notes:
  - end_input concat-einsum factorized:  einsum([seq|feat_k] @ w_end0) =
    seq @ W0a + feat_k @ W0b, so the big matmul is [S,H]@[H,H] once per
    example instead of [S,K,2H]@[2H,H].
  - LayerNorm + w_end1 dot fused algebraically:  per (s,k) only three
    H-reductions are needed: S1=sum(t), S2=sum(t^2), D=sum(t*g*w1), then
    logit = rsqrt(S2/H-(S1/H)^2+eps) * (D - S1/H*sum(g*w1)) + sum(b*w1)+b1.
  - matmuls run as float32r (TensorE full-rate fp32, 12-bit mantissa RN).
    Top-k margins verified against the rounding error via host emulation.
  - top-5 via 5x (reduce_max -> is_equal -> iota max-reduce -> mask) on DVE;
    the native max8/max_index instructions fault alongside f32r matmuls.
"""
import os
import sys
sys.path.insert(0, "/opt/trn_rl_repo")
import numpy as np

import concourse.bass as bass
import concourse.tile as tile
from concourse import bacc, mybir
from concourse.bass_types import AP
from concourse.bass_utils import run_bass_kernel_spmd

F32 = mybir.dt.float32
F32R = mybir.dt.float32r
BF16 = mybir.dt.bfloat16
I32 = mybir.dt.int32
U32 = mybir.dt.uint32
U16 = mybir.dt.uint16
AF = mybir.ActivationFunctionType
OP = mybir.AluOpType
AX = mybir.AxisListType

B, S, H = 16, 512, 1024
NCORES = 8
PEREX = B // NCORES          # 2 examples per core
K1 = 5                       # start_n_top
K2 = 5                       # end_n_top
NPAIR = PEREX * K1           # 10 (ex,k) rows
HC = H // 128                # 8 h-chunks
NEGBIG = -1e38

# consts tensor columns
C_GW, C_BW, C_BST, C_EPS, C_NGW = 0, 1, 2, 3, 4

_CACHED = {}


def build_nc():
    nc = bacc.Bacc("TRN2", target_bir_lowering=False, debug=False,
                   enable_asserts=True, num_devices=NCORES)

    # ---- inputs ----
    seqT_d = nc.dram_tensor("seqT", [PEREX, H, S], F32R, kind="ExternalInput")
    w0a_d = nc.dram_tensor("w0a", [H, H], F32R, kind="ExternalInput")
    w0b_d = nc.dram_tensor("w0b", [H, H], F32R, kind="ExternalInput")
    wans0_d = nc.dram_tensor("wans0", [2 * H, H], BF16, kind="ExternalInput")
    # pka: [128, 32] = wst(8) | onesgw(16) | b0c(8)
    pka_d = nc.dram_tensor("pka", [128, 32], F32R, kind="ExternalInput")
    # pkb: [16, 1608] = keep10(512) | negm10(512) | iota(512) | con(8) | cio(48)
    # rows: keep10/negm10 rows 0-9 (iota row 0; con rows 0-15; cio row 0)
    pkb_d = nc.dram_tensor("pkb", [16, 1608], F32, kind="ExternalInput")
    # pkc: [2, 2048] = wans1(1024) | bans0(1024)
    pkc_d = nc.dram_tensor("pkc", [PEREX, 2 * H], F32, kind="ExternalInput")

    # ---- outputs ----
    slp_d = nc.dram_tensor("slp", [PEREX, K1], F32, kind="ExternalOutput")
    sidx_d = nc.dram_tensor("sidx", [PEREX, K1], I32, kind="ExternalOutput")
    elp_d = nc.dram_tensor("elp", [PEREX, K1 * K2], F32, kind="ExternalOutput")
    eidx_d = nc.dram_tensor("eidx", [PEREX, K1 * K2], I32, kind="ExternalOutput")
    cls_d = nc.dram_tensor("cls", [PEREX, 1], F32, kind="ExternalOutput")

    with tile.TileContext(nc) as tc:
        with tc.tile_pool(name="wts", bufs=1) as wts, \
             tc.tile_pool(name="data", bufs=1) as data, \
             tc.tile_pool(name="tk", bufs=3) as tkp, \
             tc.tile_pool(name="sqk", bufs=3) as sqp, \
             tc.tile_pool(name="small", bufs=1) as sm, \
             tc.tile_pool(name="ps", bufs=7, space="PSUM") as ps:

            # ================= loads =================
            # seqT first (gates all compute), then W0A (A matmuls), then the
            # packed smalls; W0B; WANS0 reuses W0B's slot late (ACT queue).
            # SP DMA queue is in-order: load in the order compute needs it.
            # PKA (tiny, gates start-logits) -> seqT0 -> W0A half 1 (A ex0
            # m<4 can start) -> seqT1 -> W0B (gates the bias table / tanh)
            # -> W0A half 2 -> small packs.
            SEQT = []
            for ex in range(PEREX):
                t = data.tile([128, HC, S], F32R, tag=f"seqT{ex}")
                nc.sync.dma_start(t[:], seqT_d[ex].rearrange("(c p) s -> p c s", p=128))
                SEQT.append(t)
            PKA = wts.tile([128, 32], F32R, name="PKA")
            nc.sync.dma_start(PKA[:], pka_d[:])
            WST = PKA[:, 0:8]
            OGW = PKA[:, 8:24].rearrange("p (c t) -> p c t", t=2)
            B0C = PKA[:, 24:32].bitcast(F32)
            PKB = sm.tile([16, 1608], F32, name="PKB")
            nc.sync.dma_start(PKB[:], pkb_d[:])
            KEEP10 = PKB[0:NPAIR, 0:S]
            NEGM10 = PKB[0:NPAIR, S:2 * S]
            IOTA10 = PKB[0:NPAIR, 2 * S:3 * S]
            CON = PKB[0:16, 3 * S:3 * S + 8]
            CIO = PKB[0:1, 3 * S + 8:3 * S + 56]
            PKC = sm.tile([PEREX, 2 * H], F32, name="PKC")
            nc.sync.dma_start(PKC[:], pkc_d[:])
            WANS1 = PKC[:, 0:H]
            BANS0 = PKC[:, H:2 * H]
            W0A = wts.tile([128, HC, H], F32R, tag="wa", name="W0A")
            nc.sync.dma_start(W0A[:, :, 0:512],
                              w0a_d[:, 0:512].rearrange("(c p) j -> p c j", p=128))
            nc.sync.dma_start(W0A[:, :, 512:1024],
                              w0a_d[:, 512:1024].rearrange("(c p) j -> p c j", p=128))
            W0B = wts.tile([128, HC, H], F32R, tag="wbig", bufs=1, name="W0B")
            nc.sync.dma_start(W0B[:], w0b_d[:].rearrange("(c p) j -> p c j", p=128))

            # ============ start logits -> masked [2, S] ============
            # engine ops must start at partition 0: compute each example in a
            # partition-0 tile, then DMA-assemble the [2, S] stack.
            SM = sm.tile([PEREX, S], F32)
            KEEPx = [sm.tile([1, S], F32, name=f"KEEPx{e}") for e in range(PEREX)]
            NEGMx = [sm.tile([1, S], F32, name=f"NEGMx{e}") for e in range(PEREX)]
            for ex in range(PEREX):
                nc.sync.dma_start(KEEPx[ex][:], pkb_d[ex * K1:ex * K1 + 1, 0:S])
                nc.sync.dma_start(NEGMx[ex][:], pkb_d[ex * K1:ex * K1 + 1, S:2 * S])
            for ex in range(PEREX):
                psl = ps.tile([1, S], F32, tag="psm", bufs=1)
                for c in range(HC):
                    nc.tensor.matmul(psl[:], lhsT=WST[:, c:c + 1], rhs=SEQT[ex][:, c, :],
                                     start=(c == 0), stop=(c == HC - 1))
                SMx = sm.tile([1, S], F32, tag="smx", bufs=1, name=f"SMx{ex}")
                nc.vector.scalar_tensor_tensor(
                    out=SMx[:], in0=psl[:], scalar=CON[0:1, C_BST:C_BST + 1],
                    in1=KEEPx[ex][:], op0=OP.add, op1=OP.mult)
                nc.vector.tensor_tensor(out=SMx[:], in0=SMx[:],
                                        in1=NEGMx[ex][:], op=OP.add)
                nc.sync.dma_start(SM[ex:ex + 1, :], SMx[:])

            # ============ start top-5 + log-softmax + softmax ============
            NMX2 = sm.tile([PEREX, 1], F32)
            nc.vector.tensor_reduce(NMX2[:], SM[:], axis=AX.X, op=OP.max, negate=True)
            EXPS = sm.tile([PEREX, S], F32)
            SUME2 = sm.tile([PEREX, 1], F32)
            nc.scalar.activation(EXPS[:], SM[:], AF.Exp, bias=NMX2[:], scale=1.0,
                                 accum_out=SUME2[:])
            LN2 = sm.tile([PEREX, 1], F32)
            nc.scalar.activation(LN2[:], SUME2[:], AF.Ln)
            LSE2 = sm.tile([PEREX, 1], F32)
            nc.vector.tensor_tensor(out=LSE2[:], in0=LN2[:], in1=NMX2[:], op=OP.subtract)

            SV = sm.tile([PEREX, K1], F32)     # top-5 start values (masked logits)
            SIF = sm.tile([PEREX, K1], F32)    # 511 - idx
            scr2 = sm.tile([PEREX, S], F32)
            for j in range(K1):
                nc.vector.tensor_reduce(SV[:, j:j + 1], SM[:], axis=AX.X, op=OP.max)
                EQ2 = sm.tile([NPAIR, S], F32, tag="scr", bufs=3, name="EQ2")
                nc.vector.tensor_scalar(EQ2[0:PEREX, :], SM[:], SV[:, j:j + 1], None,
                                        op0=OP.is_equal)
                nc.vector.tensor_tensor(out=scr2[0:PEREX, :], in0=EQ2[0:PEREX, :],
                                        in1=IOTA10[0:PEREX, :], op=OP.mult)
                nc.vector.tensor_reduce(SIF[:, j:j + 1], scr2[0:PEREX, :],
                                        axis=AX.X, op=OP.max)
                if j < K1 - 1:
                    nc.vector.scalar_tensor_tensor(
                        out=SM[:], in0=EQ2[0:PEREX, :], scalar=NEGBIG, in1=SM[:],
                        op0=OP.mult, op1=OP.add)
            # idx = 511 - SIF ; values_out = SV - lse
            SIDXF = sm.tile([PEREX, K1], F32)
            nc.vector.tensor_scalar(SIDXF[:], SIF[:], -1.0, 511.0,
                                    op0=OP.mult, op1=OP.add)
            SIDXI = sm.tile([PEREX, K1], I32)
            nc.vector.tensor_copy(SIDXI[:], SIDXF[:])
            SLPO = sm.tile([PEREX, K1], F32)
            nc.vector.tensor_scalar(SLPO[:], SV[:], LSE2[:], None, op0=OP.subtract)
            nc.sync.dma_start(slp_d[:], SLPO[:])
            nc.sync.dma_start(sidx_d[:], SIDXI[:])

            # softmax p for the answer-class path: p = EXPS / SUME2
            REC2 = sm.tile([PEREX, 1], F32)
            nc.vector.reciprocal(REC2[:], SUME2[:])
            P2 = EXPS
            nc.vector.tensor_scalar(P2[:], EXPS[:], REC2[:], None, op0=OP.mult)

            # ============ gather start features (columns of seqT) ============
            # gpsimd.indirect_copy with wrapped u16 indices: logical index
            # i = k*8 + cc lives at idw[i%16, i//16]; its gathered value lands
            # at G2 column q = 3*(i%16) + i//16.
            G2 = sm.tile([128, PEREX, 48], F32)
            for ex in range(PEREX):
                srow = sm.tile([1, 8], F32, tag="srow", bufs=2, name=f"srow{ex}")
                nc.vector.memset(srow[:], 0.0)
                nc.sync.dma_start(srow[0:1, 0:K1], SIDXF[ex:ex + 1, :])
                vf = sm.tile([1, 48], F32, tag="vf", bufs=2, name=f"vf{ex}")
                # V[q] = srow[2*(q%3) + q//24] + cio[q]
                sview = AP(srow.tensor, srow.offset,
                           [[8, 1], [1, 2], [0, 8], [2, 3]])
                nc.vector.tensor_tensor(
                    out=vf[:].rearrange("o (a b c) -> o a b c", a=2, b=8),
                    in0=sview,
                    in1=CIO[:].rearrange("o (a b c) -> o a b c", a=2, b=8),
                    op=OP.add)
                vu = sm.tile([1, 48], U16, tag="vu", bufs=2, name=f"vu{ex}")
                nc.vector.tensor_copy(vu[:], vf[:])
                idw = sm.tile([128, 3], U16, tag="idw", bufs=2, name=f"idw{ex}")
                for g in range(8):
                    eng = (nc.scalar, nc.sync)[g % 2]
                    eng.dma_start(idw[g * 16:(g + 1) * 16, :], vu[:])
                nc.gpsimd.indirect_copy(
                    G2[:, ex, :],
                    SEQT[ex][:].bitcast(F32).rearrange("p c s -> p (c s)"),
                    idw[:], i_know_ap_gather_is_preferred=True)
            GR = sm.tile([128, PEREX, 48], F32R)
            nc.scalar.copy(GR[:], G2[:])

            # ==== B^T[h', pair] = W0b^T @ featsT  (directly transposed) ====
            # indirect_copy lands (k, cc) at G2 column q = 8*k + cc, so the
            # rhs for chunk cc is a regular [ex, k] strided view.
            BT = sm.tile([128, NPAIR, HC], F32)
            for m in range(HC):
                psb = ps.tile([128, 512], F32, tag="psm", bufs=1, name=f"psb{m}")
                psb = psb[:, 0:NPAIR]
                for cc in range(HC):
                    rhs = AP(GR.tensor, GR.offset + cc,
                             [[PEREX * 48, 128], [48, PEREX], [8, K1]])
                    nc.tensor.matmul(psb[:],
                                     lhsT=W0B[:, cc, m * 128:(m + 1) * 128].bitcast(F32),
                                     rhs=rhs.bitcast(F32),
                                     start=(cc == 0), stop=(cc == HC - 1))
                nc.vector.tensor_scalar(BT[:, :, m], psb[:], B0C[:, m:m + 1],
                                        None, op0=OP.add)

            # ============ A = seq @ W0a  (PSUM -> SBUF) ============
            ASB = []
            for ex in range(PEREX):
                a = data.tile([128, HC, S], F32, tag=f"asb{ex}")
                ASB.append(a)
                for m in range(HC):
                    pa = ps.tile([128, S], F32, tag="psa", bufs=3)
                    for c in range(HC):
                        nc.tensor.matmul(pa[:], lhsT=W0A[:, c, m * 128:(m + 1) * 128],
                                         rhs=SEQT[ex][:, c, :],
                                         start=(c == 0), stop=(c == HC - 1))
                    nc.scalar.copy(a[:, m, :], pa[:])

            # ============ tanh / square / reductions ============
            SDT = sm.tile([NPAIR, 3 * S], F32)
            S1T = SDT[:, 0:S]
            DT = SDT[:, S:2 * S]
            S2T = SDT[:, 2 * S:3 * S]
            for ex in range(PEREX):
                for k in range(K1):
                    pair = ex * K1 + k
                    pr1 = ps.tile([2, S], F32, tag="psr1", bufs=2)
                    pr2 = ps.tile([1, S], F32, tag="psr2", bufs=2)
                    for m in range(HC):
                        t = tkp.tile([128, S], F32R, tag="t")
                        nc.scalar.activation(t[:], ASB[ex][:, m, :], AF.Tanh,
                                             bias=BT[:, pair, m:m + 1], scale=1.0)
                        sq = sqp.tile([128, S], F32R, tag="sq")
                        eng = (nc.vector, nc.gpsimd)[m % 2]
                        eng.tensor_tensor(out=sq[:], in0=t[:].bitcast(F32),
                                          in1=t[:].bitcast(F32), op=OP.mult)
                        nc.tensor.matmul(pr1[:], lhsT=OGW[:, m, :], rhs=t[:],
                                         start=(m == 0), stop=(m == HC - 1))
                        nc.tensor.matmul(pr2[:], lhsT=OGW[:, m, 0:1], rhs=sq[:],
                                         start=(m == 0), stop=(m == HC - 1))
                    ev1 = sqp.tile([2, S], F32, tag="ev1", bufs=2, name=f"ev1_{pair}")
                    nc.scalar.copy(ev1[:], pr1[:])
                    ev2 = sqp.tile([1, S], F32, tag="ev2", bufs=2, name=f"ev2_{pair}")
                    nc.vector.tensor_copy(ev2[:], pr2[:])
                    eng = (nc.scalar, nc.sync)[pair % 2]
                    eng.dma_start(SDT[pair:pair + 1, 0:2 * S], ev1[:])
                    eng.dma_start(SDT[pair:pair + 1, 2 * S:3 * S], ev2[:])

            # ============ fused LN epilogue -> masked end logits [10, S] ========
            invH = 1.0 / H
            MU = sm.tile([NPAIR, S], F32, tag="scr", bufs=3, name="MU")
            nc.vector.tensor_scalar(MU[:], S1T[:], invH, None, op0=OP.mult)
            MU2 = sm.tile([NPAIR, S], F32, tag="scr", bufs=3, name="MU2")
            nc.gpsimd.tensor_tensor(out=MU2[:], in0=MU[:], in1=MU[:], op=OP.mult)
            VAR = sm.tile([NPAIR, S], F32, tag="scr", bufs=3, name="VAR")
            nc.vector.scalar_tensor_tensor(out=VAR[:], in0=S2T[:], scalar=invH,
                                           in1=MU2[:], op0=OP.mult, op1=OP.subtract)
            SD = sm.tile([NPAIR, S], F32, tag="scr", bufs=3, name="SD")
            nc.scalar.activation(SD[:], VAR[:], AF.Sqrt,
                                 bias=CON[0:NPAIR, C_EPS:C_EPS + 1], scale=1.0)
            RR = sm.tile([NPAIR, S], F32, tag="scr", bufs=3, name="RR")
            nc.vector.reciprocal(RR[:], SD[:])
            # X = D - mu*c_gw   via  (mu * -c_gw) + D
            X = sm.tile([NPAIR, S], F32, tag="scr", bufs=3, name="X")
            nc.vector.scalar_tensor_tensor(out=X[:], in0=MU[:],
                                           scalar=CON[0:NPAIR, C_NGW:C_NGW + 1],
                                           in1=DT[:], op0=OP.mult, op1=OP.add)
            EM = sm.tile([NPAIR, S], F32)
            nc.gpsimd.tensor_tensor(out=EM[:], in0=X[:], in1=RR[:], op=OP.mult)
            Z = sm.tile([NPAIR, S], F32, tag="scr", bufs=3, name="Z")
            nc.vector.scalar_tensor_tensor(out=Z[:], in0=EM[:],
                                           scalar=CON[0:NPAIR, C_BW:C_BW + 1],
                                           in1=KEEP10[:], op0=OP.add, op1=OP.mult)
            nc.vector.tensor_tensor(out=EM[:], in0=Z[:], in1=NEGM10[:], op=OP.add)

            # ============ end log-softmax + top-5 ============
            NMX10 = sm.tile([NPAIR, 1], F32)
            nc.vector.tensor_reduce(NMX10[:], EM[:], axis=AX.X, op=OP.max, negate=True)
            EXPE = sm.tile([NPAIR, S], F32, tag="scr", bufs=3, name="EXPE")
            SUME10 = sm.tile([NPAIR, 1], F32)
            nc.scalar.activation(EXPE[:], EM[:], AF.Exp, bias=NMX10[:], scale=1.0,
                                 accum_out=SUME10[:])
            LN10 = sm.tile([NPAIR, 1], F32)
            nc.scalar.activation(LN10[:], SUME10[:], AF.Ln)
            LSE10 = sm.tile([NPAIR, 1], F32)
            nc.vector.tensor_tensor(out=LSE10[:], in0=LN10[:], in1=NMX10[:],
                                    op=OP.subtract)

            EV = sm.tile([NPAIR, K2], F32)
            EIF = sm.tile([NPAIR, K2], F32)
            scr10 = sm.tile([NPAIR, S], F32, tag="scr", bufs=3, name="scr10")
            for j in range(K2):
                nc.vector.tensor_reduce(EV[:, j:j + 1], EM[:], axis=AX.X, op=OP.max)
                EQ10 = sm.tile([NPAIR, S], F32, tag="scr", bufs=3, name="EQ10")
                nc.vector.tensor_scalar(EQ10[:], EM[:], EV[:, j:j + 1], None,
                                        op0=OP.is_equal)
                nc.vector.tensor_tensor(out=scr10[:], in0=EQ10[:],
                                         in1=IOTA10[:], op=OP.mult)
                nc.vector.tensor_reduce(EIF[:, j:j + 1], scr10[:],
                                        axis=AX.X, op=OP.max)
                if j < K2 - 1:
                    nc.vector.scalar_tensor_tensor(
                        out=EM[:], in0=EQ10[:], scalar=NEGBIG, in1=EM[:],
                        op0=OP.mult, op1=OP.add)
            EIDXF = sm.tile([NPAIR, K2], F32)
            nc.vector.tensor_scalar(EIDXF[:], EIF[:], -1.0, 511.0,
                                    op0=OP.mult, op1=OP.add)
            EIDXI = sm.tile([NPAIR, K2], I32)
            nc.vector.tensor_copy(EIDXI[:], EIDXF[:])
            ELPO = sm.tile([NPAIR, K2], F32)
            nc.vector.tensor_scalar(ELPO[:], EV[:], LSE10[:], None, op0=OP.subtract)
            nc.sync.dma_start(elp_d[:].rearrange("b (k j) -> (b k) j", j=K2), ELPO[:])
            nc.sync.dma_start(eidx_d[:].rearrange("b (k j) -> (b k) j", j=K2), EIDXI[:])

            # ============ answer-class path ============
            FEATC = sm.tile([128, 2 * HC, PEREX], F32)
            scrF = sm.tile([128, S], F32, tag="scr", bufs=3, name="scrF")
            PB = sm.tile([128, S], F32, tag="scr", bufs=3, name="PB")
            P2x = sm.tile([NPAIR, S], F32, tag="scr", bufs=3, name="P2x")
            for ex in range(PEREX):
                if ex == 0:
                    nc.gpsimd.partition_broadcast(PB[:], P2[0:1, :])
                else:
                    nc.sync.dma_start(P2x[0:1, :], P2[ex:ex + 1, :])
                    nc.gpsimd.partition_broadcast(PB[:], P2x[0:1, :])
                for c in range(HC):
                    nc.vector.tensor_tensor(out=scrF[:], in0=SEQT[ex][:, c, :].bitcast(F32),
                                            in1=PB[:], op=OP.mult)
                    nc.vector.tensor_reduce(FEATC[:, c, ex:ex + 1], scrF[:],
                                            axis=AX.X, op=OP.add)
                for c in range(HC):
                    nc.vector.tensor_copy(FEATC[:, HC + c, ex:ex + 1],
                                          SEQT[ex][:, c, 0:1].bitcast(F32))
            WANS0 = wts.tile([128, 2 * HC, H], BF16, tag="wbig", bufs=1, name="WANS0")
            nc.sync.dma_start(WANS0[:], wans0_d[:].rearrange("(c p) j -> p c j", p=128))
            FB = sm.tile([128, 2 * HC, PEREX], BF16)
            nc.vector.tensor_copy(FB[:], FEATC[:])
            psa = [ps.tile([PEREX, 512], F32, tag="psm", bufs=1, name=f"psa{i}") for i in range(2)]
            for nchunk in range(2):
                for c in range(2 * HC):
                    nc.tensor.matmul(psa[nchunk][:], lhsT=FB[:, c, :],
                                     rhs=WANS0[:, c, nchunk * 512:(nchunk + 1) * 512],
                                     start=(c == 0), stop=(c == 2 * HC - 1))
            ANS = sm.tile([PEREX, H], F32, tag="b4k", bufs=3, name="ANS")
            nc.vector.tensor_tensor(out=ANS[:, 0:512], in0=psa[0][:],
                                    in1=BANS0[:, 0:512], op=OP.add)
            nc.vector.tensor_tensor(out=ANS[:, 512:1024], in0=psa[1][:],
                                    in1=BANS0[:, 512:1024], op=OP.add)
            TANS = sm.tile([PEREX, H], F32, tag="b4k", bufs=3, name="ANS")
            nc.scalar.activation(TANS[:], ANS[:], AF.Tanh)
            scrA = sm.tile([PEREX, H], F32, tag="b4k", bufs=3, name="scrA")
            CLS = sm.tile([PEREX, 1], F32)
            nc.vector.tensor_tensor(out=scrA[:], in0=TANS[:], in1=WANS1[:], op=OP.mult)
            nc.vector.tensor_reduce(CLS[:], scrA[:], axis=AX.X, op=OP.add)
            nc.sync.dma_start(cls_d[:], CLS[:])

    nc.compile()
    return nc


def _host_prep(inputs):
    seq = np.asarray(inputs["sequence_output"], np.float32)
    pm = np.asarray(inputs["p_mask"], np.float32)
    w_start = np.asarray(inputs["w_start"], np.float32)[:, 0]
    b_start = float(np.asarray(inputs["b_start"], np.float32)[0])
    w_end0 = np.asarray(inputs["w_end0"], np.float32)
    b_end0 = np.asarray(inputs["b_end0"], np.float32)
    ln_g = np.asarray(inputs["ln_g"], np.float32)
    ln_b = np.asarray(inputs["ln_b"], np.float32)
    w_end1 = np.asarray(inputs["w_end1"], np.float32)[:, 0]
    b_end1 = float(np.asarray(inputs["b_end1"], np.float32)[0])
    w_ans0 = np.asarray(inputs["w_ans0"], np.float32)
    b_ans0 = np.asarray(inputs["b_ans0"], np.float32)
    w_ans1 = np.asarray(inputs["w_ans1"], np.float32)[:, 0]

    import ml_dtypes
    gw = (np.float64(ln_g) * np.float64(w_end1)).astype(np.float32)
    c_gw = float(np.float64(gw).sum())
    c_bw = float((np.float64(ln_b) * np.float64(w_end1)).sum() + b_end1)

    # pka: [128, 32] = wst(8) | onesgw(16, interleaved [c][2]) | b0c(8)
    pka = np.zeros((128, 32), np.float32)
    pka[:, 0:8] = w_start.reshape(HC, 128).T
    og = np.stack([np.ones((128, HC), np.float32), gw.reshape(HC, 128).T],
                  axis=2)                      # [128, HC, 2]
    pka[:, 8:24] = og.reshape(128, 16)
    pka[:, 24:32] = b_end0.reshape(HC, 128).T

    consts = np.zeros((16, 8), np.float32)
    consts[:, C_GW] = c_gw
    consts[:, C_BW] = c_bw
    consts[:, C_BST] = b_start
    consts[:, C_EPS] = 1e-12
    consts[:, C_NGW] = -c_gw
    cio = np.zeros(48, np.float32)
    for q in range(48):
        cio[q] = 512.0 * ((q // 3) % 8)
    iota = (float(S - 1) - np.arange(S, dtype=np.float32))

    common = {
        "w0a": np.ascontiguousarray(w_end0[:H]),
        "w0b": np.ascontiguousarray(w_end0[H:]),
        "wans0": w_ans0.astype(ml_dtypes.bfloat16),
        "pka": pka,
        "pkc": np.concatenate([np.broadcast_to(w_ans1, (PEREX, H)),
                               np.broadcast_to(b_ans0, (PEREX, H))],
                              axis=1).astype(np.float32),
    }

    in_maps = []
    for core in range(NCORES):
        sl = slice(core * PEREX, (core + 1) * PEREX)
        keep = (1.0 - pm[sl]).astype(np.float32)
        negm = (-1e30 * pm[sl]).astype(np.float32)
        pkb = np.zeros((16, 1608), np.float32)
        pkb[0:NPAIR, 0:S] = np.repeat(keep, K1, axis=0)
        pkb[0:NPAIR, S:2 * S] = np.repeat(negm, K1, axis=0)
        pkb[0:NPAIR, 2 * S:3 * S] = iota[None, :]
        pkb[0:16, 3 * S:3 * S + 8] = consts
        pkb[0:1, 3 * S + 8:3 * S + 56] = cio[None, :]
        m = dict(common)
        m["seqT"] = np.ascontiguousarray(seq[sl].transpose(0, 2, 1))
        m["pkb"] = pkb
        in_maps.append(m)
    return in_maps


def kernel(**inputs):
    assert int(inputs["start_n_top"]) == K1 and int(inputs["end_n_top"]) == K2
    if "nc" not in _CACHED:
        _CACHED["nc"] = build_nc()
    nc = _CACHED["nc"]
    in_maps = _host_prep(inputs)
    trace = os.environ.get("KERNEL_TRACE", "") == "1"
    res = run_bass_kernel_spmd(nc, in_maps, core_ids=list(range(NCORES)),
                               trace=trace)
    _CACHED["last_result"] = res
    rs = res.results
    slp = np.concatenate([r["slp"] for r in rs], axis=0)
    sidx = np.concatenate([r["sidx"] for r in rs], axis=0).astype(np.int32)
    elp = np.concatenate([r["elp"] for r in rs], axis=0)
    eidx = np.concatenate([r["eidx"] for r in rs], axis=0).astype(np.int32)
    cls = np.concatenate([r["cls"] for r in rs], axis=0)[:, 0]
    return slp, sidx, elp, eidx, cls


# revision 38
# speedup vs baseline: 1.0115x; 1.0115x over previous
"""ALBertQALayer beam-search head on 8 Trainium2 NeuronCores.

Data-parallel: B=16 examples sharded 2-per-core; all params replicated.
Math notes:
  - end_input concat-einsum factorized:  einsum([seq|feat_k] @ w_end0) =
    seq @ W0a + feat_k @ W0b, so the big matmul is [S,H]@[H,H] once per
    example instead of [S,K,2H]@[2H,H].
  - LayerNorm + w_end1 dot fused algebraically:  per (s,k) only three
    H-reductions are needed: S1=sum(t), S2=sum(t^2), D=sum(t*g*w1), then
    logit = rsqrt(S2/H-(S1/H)^2+eps) * (D - S1/H*sum(g*w1)) + sum(b*w1)+b1.
  - matmuls run as float32r (TensorE full-rate fp32, 12-bit mantissa RN).
    Top-k margins verified against the rounding error via host emulation.
  - top-5 via 5x (reduce_max -> is_equal -> iota max-reduce -> mask) on DVE;
    the native max8/max_index instructions fault alongside f32r matmuls.
"""
import os
import sys
sys.path.insert(0, "/opt/trn_rl_repo")
import numpy as np

import concourse.bass as bass
import concourse.tile as tile
from concourse import bacc, mybir
from concourse.bass_types import AP
from concourse.bass_utils import run_bass_kernel_spmd

F32 = mybir.dt.float32
F32R = mybir.dt.float32r
BF16 = mybir.dt.bfloat16
I32 = mybir.dt.int32
U32 = mybir.dt.uint32
U16 = mybir.dt.uint16
AF = mybir.ActivationFunctionType
OP = mybir.AluOpType
AX = mybir.AxisListType

B, S, H = 16, 512, 1024
NCORES = 8
PEREX = B // NCORES          # 2 examples per core
K1 = 5                       # start_n_top
K2 = 5                       # end_n_top
NPAIR = PEREX * K1           # 10 (ex,k) rows
HC = H // 128                # 8 h-chunks
NEGBIG = -1e38

# consts tensor columns
C_GW, C_BW, C_BST, C_EPS, C_NGW = 0, 1, 2, 3, 4

_CACHED = {}


def build_nc():
    nc = bacc.Bacc("TRN2", target_bir_lowering=False, debug=False,
                   enable_asserts=True, num_devices=NCORES)

    # ---- inputs ----
    seqT_d = nc.dram_tensor("seqT", [PEREX, H, S], F32R, kind="ExternalInput")
    w0a_d = nc.dram_tensor("w0a", [H, H], F32R, kind="ExternalInput")
    w0b_d = nc.dram_tensor("w0b", [H, H], F32R, kind="ExternalInput")
    wans0_d = nc.dram_tensor("wans0", [2 * H, H], BF16, kind="ExternalInput")
    # pka: [128, 32] = wst(8) | onesgw(16) | b0c(8)
    pka_d = nc.dram_tensor("pka", [128, 32], F32R, kind="ExternalInput")
    # pkb: [16, 1608] = keep10(512) | negm10(512) | iota(512) | con(8) | cio(48)
    # rows: keep10/negm10 rows 0-9 (iota row 0; con rows 0-15; cio row 0)
    pkb_d = nc.dram_tensor("pkb", [16, 1608], F32, kind="ExternalInput")
    # pkc: [2, 2048] = wans1(1024) | bans0(1024)
    pkc_d = nc.dram_tensor("pkc", [PEREX, 2 * H], F32, kind="ExternalInput")

    # ---- outputs ----
    slp_d = nc.dram_tensor("slp", [PEREX, K1], F32, kind="ExternalOutput")
    sidx_d = nc.dram_tensor("sidx", [PEREX, K1], I32, kind="ExternalOutput")
    elp_d = nc.dram_tensor("elp", [PEREX, K1 * K2], F32, kind="ExternalOutput")
    eidx_d = nc.dram_tensor("eidx", [PEREX, K1 * K2], I32, kind="ExternalOutput")
    cls_d = nc.dram_tensor("cls", [PEREX, 1], F32, kind="ExternalOutput")

    with tile.TileContext(nc) as tc:
        with tc.tile_pool(name="wts", bufs=1) as wts, \
             tc.tile_pool(name="data", bufs=1) as data, \
             tc.tile_pool(name="tk", bufs=3) as tkp, \
             tc.tile_pool(name="sqk", bufs=3) as sqp, \
             tc.tile_pool(name="small", bufs=1) as sm, \
             tc.tile_pool(name="ps", bufs=7, space="PSUM") as ps:

            # ================= loads =================
            # seqT first (gates all compute), then W0A (A matmuls), then the
            # packed smalls; W0B; WANS0 reuses W0B's slot late (ACT queue).
            # SP DMA queue is in-order: load in the order compute needs it.
            # PKA (tiny, gates start-logits) -> seqT0 -> W0A half 1 (A ex0
            # m<4 can start) -> seqT1 -> W0B (gates the bias table / tanh)
            # -> W0A half 2 -> small packs.
            SEQT = []
            for ex in range(PEREX):
                t = data.tile([128, HC, S], F32R, tag=f"seqT{ex}")
                nc.sync.dma_start(t[:], seqT_d[ex].rearrange("(c p) s -> p c s", p=128))
                SEQT.append(t)
            PKA = wts.tile([128, 32], F32R, name="PKA")
            nc.sync.dma_start(PKA[:], pka_d[:])
            WST = PKA[:, 0:8]
            OGW = PKA[:, 8:24].rearrange("p (c t) -> p c t", t=2)
            B0C = PKA[:, 24:32].bitcast(F32)
            PKB = sm.tile([16, 1608], F32, name="PKB")
            nc.sync.dma_start(PKB[:], pkb_d[:])
            KEEP10 = PKB[0:NPAIR, 0:S]
            NEGM10 = PKB[0:NPAIR, S:2 * S]
            IOTA10 = PKB[0:NPAIR, 2 * S:3 * S]
            CON = PKB[0:16, 3 * S:3 * S + 8]
            CIO = PKB[0:1, 3 * S + 8:3 * S + 56]
            PKC = sm.tile([PEREX, 2 * H], F32, name="PKC")
            nc.sync.dma_start(PKC[:], pkc_d[:])
            WANS1 = PKC[:, 0:H]
            BANS0 = PKC[:, H:2 * H]
            W0A = wts.tile([128, HC, H], F32R, tag="wa", name="W0A")
            nc.sync.dma_start(W0A[:, :, 0:512],
                              w0a_d[:, 0:512].rearrange("(c p) j -> p c j", p=128))
            nc.sync.dma_start(W0A[:, :, 512:1024],
                              w0a_d[:, 512:1024].rearrange("(c p) j -> p c j", p=128))
            W0B = wts.tile([128, HC, H], F32R, tag="wbig", bufs=1, name="W0B")
            nc.sync.dma_start(W0B[:], w0b_d[:].rearrange("(c p) j -> p c j", p=128))

            # ============ start logits -> masked [2, S] ============
            # engine ops must start at partition 0: compute each example in a
            # partition-0 tile, then DMA-assemble the [2, S] stack.
            SM = sm.tile([PEREX, S], F32)
            KEEPx = [sm.tile([1, S], F32, name=f"KEEPx{e}") for e in range(PEREX)]
            NEGMx = [sm.tile([1, S], F32, name=f"NEGMx{e}") for e in range(PEREX)]
            for ex in range(PEREX):
                nc.sync.dma_start(KEEPx[ex][:], pkb_d[ex * K1:ex * K1 + 1, 0:S])
                nc.sync.dma_start(NEGMx[ex][:], pkb_d[ex * K1:ex * K1 + 1, S:2 * S])
            for ex in range(PEREX):
                psl = ps.tile([1, S], F32, tag="psm", bufs=1)
                for c in range(HC):
                    nc.tensor.matmul(psl[:], lhsT=WST[:, c:c + 1], rhs=SEQT[ex][:, c, :],
                                     start=(c == 0), stop=(c == HC - 1))
                SMx = sm.tile([1, S], F32, tag="smx", bufs=1, name=f"SMx{ex}")
                nc.vector.scalar_tensor_tensor(
                    out=SMx[:], in0=psl[:], scalar=CON[0:1, C_BST:C_BST + 1],
                    in1=KEEPx[ex][:], op0=OP.add, op1=OP.mult)
                nc.vector.tensor_tensor(out=SMx[:], in0=SMx[:],
                                        in1=NEGMx[ex][:], op=OP.add)
                nc.sync.dma_start(SM[ex:ex + 1, :], SMx[:])

            # ============ start top-5 + log-softmax + softmax ============
            NMX2 = sm.tile([PEREX, 1], F32)
            nc.vector.tensor_reduce(NMX2[:], SM[:], axis=AX.X, op=OP.max, negate=True)
            EXPS = sm.tile([PEREX, S], F32)
            SUME2 = sm.tile([PEREX, 1], F32)
            nc.scalar.activation(EXPS[:], SM[:], AF.Exp, bias=NMX2[:], scale=1.0,
                                 accum_out=SUME2[:])
            LN2 = sm.tile([PEREX, 1], F32)
            nc.scalar.activation(LN2[:], SUME2[:], AF.Ln)
            LSE2 = sm.tile([PEREX, 1], F32)
            nc.vector.tensor_tensor(out=LSE2[:], in0=LN2[:], in1=NMX2[:], op=OP.subtract)

            SV = sm.tile([PEREX, K1], F32)     # top-5 start values (masked logits)
            SIF = sm.tile([PEREX, K1], F32)    # 511 - idx
            scr2 = sm.tile([PEREX, S], F32)
            for j in range(K1):
                nc.vector.tensor_reduce(SV[:, j:j + 1], SM[:], axis=AX.X, op=OP.max)
                EQ2 = sm.tile([NPAIR, S], F32, tag="scr", bufs=3, name="EQ2")
                nc.vector.tensor_scalar(EQ2[0:PEREX, :], SM[:], SV[:, j:j + 1], None,
                                        op0=OP.is_equal)
                nc.vector.tensor_tensor(out=scr2[0:PEREX, :], in0=EQ2[0:PEREX, :],
                                        in1=IOTA10[0:PEREX, :], op=OP.mult)
                nc.vector.tensor_reduce(SIF[:, j:j + 1], scr2[0:PEREX, :],
                                        axis=AX.X, op=OP.max)
                if j < K1 - 1:
                    nc.vector.scalar_tensor_tensor(
                        out=SM[:], in0=EQ2[0:PEREX, :], scalar=NEGBIG, in1=SM[:],
                        op0=OP.mult, op1=OP.add)
            # idx = 511 - SIF ; values_out = SV - lse
            SIDXF = sm.tile([PEREX, K1], F32)
            nc.vector.tensor_scalar(SIDXF[:], SIF[:], -1.0, 511.0,
                                    op0=OP.mult, op1=OP.add)
            SIDXI = sm.tile([PEREX, K1], I32)
            nc.vector.tensor_copy(SIDXI[:], SIDXF[:])
            SLPO = sm.tile([PEREX, K1], F32)
            nc.vector.tensor_scalar(SLPO[:], SV[:], LSE2[:], None, op0=OP.subtract)
            nc.sync.dma_start(slp_d[:], SLPO[:])
            nc.sync.dma_start(sidx_d[:], SIDXI[:])

            # softmax p for the answer-class path: p = EXPS / SUME2
            REC2 = sm.tile([PEREX, 1], F32)
            nc.vector.reciprocal(REC2[:], SUME2[:])
            P2 = EXPS
            nc.vector.tensor_scalar(P2[:], EXPS[:], REC2[:], None, op0=OP.mult)

            # ============ gather start features (columns of seqT) ============
            # gpsimd.indirect_copy with wrapped u16 indices: logical index
            # i = k*8 + cc lives at idw[i%16, i//16]; its gathered value lands
            # at G2 column q = 3*(i%16) + i//16.
            G2 = sm.tile([128, PEREX, 48], F32)
            for ex in range(PEREX):
                srow = sm.tile([1, 8], F32, tag="srow", bufs=2, name=f"srow{ex}")
                nc.vector.memset(srow[:], 0.0)
                nc.sync.dma_start(srow[0:1, 0:K1], SIDXF[ex:ex + 1, :])
                vf = sm.tile([1, 48], F32, tag="vf", bufs=2, name=f"vf{ex}")
                # V[q] = srow[2*(q%3) + q//24] + cio[q]
                sview = AP(srow.tensor, srow.offset,
                           [[8, 1], [1, 2], [0, 8], [2, 3]])
                nc.vector.tensor_tensor(
                    out=vf[:].rearrange("o (a b c) -> o a b c", a=2, b=8),
                    in0=sview,
                    in1=CIO[:].rearrange("o (a b c) -> o a b c", a=2, b=8),
                    op=OP.add)
                vu = sm.tile([1, 48], U16, tag="vu", bufs=2, name=f"vu{ex}")
                nc.vector.tensor_copy(vu[:], vf[:])
                idw = sm.tile([128, 3], U16, tag="idw", bufs=2, name=f"idw{ex}")
                for g in range(8):
                    eng = (nc.scalar, nc.sync)[g % 2]
                    eng.dma_start(idw[g * 16:(g + 1) * 16, :], vu[:])
                nc.gpsimd.indirect_copy(
                    G2[:, ex, :],
                    SEQT[ex][:].bitcast(F32).rearrange("p c s -> p (c s)"),
                    idw[:], i_know_ap_gather_is_preferred=True)
            GR = sm.tile([128, PEREX, 48], F32R)
            nc.scalar.copy(GR[:], G2[:])

            # ==== B^T[h', pair] = W0b^T @ featsT  (directly transposed) ====
            # indirect_copy lands (k, cc) at G2 column q = 8*k + cc, so the
            # rhs for chunk cc is a regular [ex, k] strided view.
            BT = sm.tile([128, NPAIR, HC], F32)
            for m in range(HC):
                psb = ps.tile([128, 512], F32, tag="psm", bufs=1, name=f"psb{m}")
                psb = psb[:, 0:NPAIR]
                for cc in range(HC):
                    rhs = AP(GR.tensor, GR.offset + cc,
                             [[PEREX * 48, 128], [48, PEREX], [8, K1]])
                    nc.tensor.matmul(psb[:],
                                     lhsT=W0B[:, cc, m * 128:(m + 1) * 128].bitcast(F32),
                                     rhs=rhs.bitcast(F32),
                                     start=(cc == 0), stop=(cc == HC - 1))
                nc.vector.tensor_scalar(BT[:, :, m], psb[:], B0C[:, m:m + 1],
                                        None, op0=OP.add)

            # ============ A = seq @ W0a  (PSUM -> SBUF) ============
            ASB = []
            for ex in range(PEREX):
                a = data.tile([128, HC, S], F32, tag=f"asb{ex}")
                ASB.append(a)
                for m in range(HC):
                    pa = ps.tile([128, S], F32, tag="psa", bufs=2)
                    for c in range(HC):
                        nc.tensor.matmul(pa[:], lhsT=W0A[:, c, m * 128:(m + 1) * 128],
                                         rhs=SEQT[ex][:, c, :],
                                         start=(c == 0), stop=(c == HC - 1))
                    nc.scalar.copy(a[:, m, :], pa[:])

            # ============ tanh / square / reductions ============
            SDT = sm.tile([NPAIR, 3 * S], F32)
            S1T = SDT[:, 0:S]
            DT = SDT[:, S:2 * S]
            S2T = SDT[:, 2 * S:3 * S]
            for ex in range(PEREX):
                for k in range(K1):
                    pair = ex * K1 + k
                    pr1 = ps.tile([2, S], F32, tag="psr1", bufs=3)
                    pr2 = ps.tile([1, S], F32, tag="psr2", bufs=2)
                    for m in range(HC):
                        t = tkp.tile([128, S], F32R, tag="t")
                        nc.scalar.activation(t[:], ASB[ex][:, m, :], AF.Tanh,
                                             bias=BT[:, pair, m:m + 1], scale=1.0)
                        sq = sqp.tile([128, S], F32R, tag="sq")
                        eng = (nc.vector, nc.vector, nc.vector, nc.gpsimd)[m % 4]
                        eng.tensor_tensor(out=sq[:], in0=t[:].bitcast(F32),
                                          in1=t[:].bitcast(F32), op=OP.mult)
                        nc.tensor.matmul(pr1[:], lhsT=OGW[:, m, :], rhs=t[:],
                                         start=(m == 0), stop=(m == HC - 1))
                        nc.tensor.matmul(pr2[:], lhsT=OGW[:, m, 0:1], rhs=sq[:],
                                         start=(m == 0), stop=(m == HC - 1))
                    ev1 = sqp.tile([2, S], F32, tag="ev1", bufs=2, name=f"ev1_{pair}")
                    nc.scalar.copy(ev1[:], pr1[:])
                    ev2 = sqp.tile([1, S], F32, tag="ev2", bufs=2, name=f"ev2_{pair}")
                    nc.vector.tensor_copy(ev2[:], pr2[:])
                    eng = (nc.scalar, nc.sync)[pair % 2]
                    eng.dma_start(SDT[pair:pair + 1, 0:2 * S], ev1[:])
                    eng.dma_start(SDT[pair:pair + 1, 2 * S:3 * S], ev2[:])

            # ============ fused LN epilogue -> masked end logits [10, S] ========
            invH = 1.0 / H
            MU = sm.tile([NPAIR, S], F32, tag="scr", bufs=3, name="MU")
            nc.vector.tensor_scalar(MU[:], S1T[:], invH, None, op0=OP.mult)
            MU2 = sm.tile([NPAIR, S], F32, tag="scr", bufs=3, name="MU2")
            nc.gpsimd.tensor_tensor(out=MU2[:], in0=MU[:], in1=MU[:], op=OP.mult)
            VAR = sm.tile([NPAIR, S], F32, tag="scr", bufs=3, name="VAR")
            nc.vector.scalar_tensor_tensor(out=VAR[:], in0=S2T[:], scalar=invH,
                                           in1=MU2[:], op0=OP.mult, op1=OP.subtract)
            SD = sm.tile([NPAIR, S], F32, tag="scr", bufs=3, name="SD")
            nc.scalar.activation(SD[:], VAR[:], AF.Sqrt,
                                 bias=CON[0:NPAIR, C_EPS:C_EPS + 1], scale=1.0)
            RR = sm.tile([NPAIR, S], F32, tag="scr", bufs=3, name="RR")
            nc.vector.reciprocal(RR[:], SD[:])
            # X = D - mu*c_gw   via  (mu * -c_gw) + D
            X = sm.tile([NPAIR, S], F32, tag="scr", bufs=3, name="X")
            nc.vector.scalar_tensor_tensor(out=X[:], in0=MU[:],
                                           scalar=CON[0:NPAIR, C_NGW:C_NGW + 1],
                                           in1=DT[:], op0=OP.mult, op1=OP.add)
            EM = sm.tile([NPAIR, S], F32)
            nc.gpsimd.tensor_tensor(out=EM[:], in0=X[:], in1=RR[:], op=OP.mult)
            Z = sm.tile([NPAIR, S], F32, tag="scr", bufs=3, name="Z")
            nc.vector.scalar_tensor_tensor(out=Z[:], in0=EM[:],
                                           scalar=CON[0:NPAIR, C_BW:C_BW + 1],
                                           in1=KEEP10[:], op0=OP.add, op1=OP.mult)
            nc.vector.tensor_tensor(out=EM[:], in0=Z[:], in1=NEGM10[:], op=OP.add)

            # ============ end log-softmax + top-5 ============
            NMX10 = sm.tile([NPAIR, 1], F32)
            nc.vector.tensor_reduce(NMX10[:], EM[:], axis=AX.X, op=OP.max, negate=True)
            EXPE = sm.tile([NPAIR, S], F32, tag="scr", bufs=3, name="EXPE")
            SUME10 = sm.tile([NPAIR, 1], F32)
            nc.scalar.activation(EXPE[:], EM[:], AF.Exp, bias=NMX10[:], scale=1.0,
                                 accum_out=SUME10[:])
            LN10 = sm.tile([NPAIR, 1], F32)
            nc.scalar.activation(LN10[:], SUME10[:], AF.Ln)
            LSE10 = sm.tile([NPAIR, 1], F32)
            nc.vector.tensor_tensor(out=LSE10[:], in0=LN10[:], in1=NMX10[:],
                                    op=OP.subtract)

            EV = sm.tile([NPAIR, K2], F32)
            EIF = sm.tile([NPAIR, K2], F32)
            scr10 = sm.tile([NPAIR, S], F32, tag="scr", bufs=3, name="scr10")
            for j in range(K2):
                nc.vector.tensor_reduce(EV[:, j:j + 1], EM[:], axis=AX.X, op=OP.max)
                EQ10 = sm.tile([NPAIR, S], F32, tag="scr", bufs=3, name="EQ10")
                nc.vector.tensor_scalar(EQ10[:], EM[:], EV[:, j:j + 1], None,
                                        op0=OP.is_equal)
                nc.vector.tensor_tensor(out=scr10[:], in0=EQ10[:],
                                         in1=IOTA10[:], op=OP.mult)
                nc.vector.tensor_reduce(EIF[:, j:j + 1], scr10[:],
                                        axis=AX.X, op=OP.max)
                if j < K2 - 1:
                    nc.vector.scalar_tensor_tensor(
                        out=EM[:], in0=EQ10[:], scalar=NEGBIG, in1=EM[:],
                        op0=OP.mult, op1=OP.add)
            EIDXF = sm.tile([NPAIR, K2], F32)
            nc.vector.tensor_scalar(EIDXF[:], EIF[:], -1.0, 511.0,
                                    op0=OP.mult, op1=OP.add)
            EIDXI = sm.tile([NPAIR, K2], I32)
            nc.vector.tensor_copy(EIDXI[:], EIDXF[:])
            ELPO = sm.tile([NPAIR, K2], F32)
            nc.vector.tensor_scalar(ELPO[:], EV[:], LSE10[:], None, op0=OP.subtract)
            nc.sync.dma_start(elp_d[:].rearrange("b (k j) -> (b k) j", j=K2), ELPO[:])
            nc.sync.dma_start(eidx_d[:].rearrange("b (k j) -> (b k) j", j=K2), EIDXI[:])

            # ============ answer-class path ============
            FEATC = sm.tile([128, 2 * HC, PEREX], F32)
            scrF = sm.tile([128, S], F32, tag="scr", bufs=3, name="scrF")
            PB = sm.tile([128, S], F32, tag="scr", bufs=3, name="PB")
            P2x = sm.tile([NPAIR, S], F32, tag="scr", bufs=3, name="P2x")
            for ex in range(PEREX):
                if ex == 0:
                    nc.gpsimd.partition_broadcast(PB[:], P2[0:1, :])
                else:
                    nc.sync.dma_start(P2x[0:1, :], P2[ex:ex + 1, :])
                    nc.gpsimd.partition_broadcast(PB[:], P2x[0:1, :])
                for c in range(HC):
                    nc.vector.tensor_tensor(out=scrF[:], in0=SEQT[ex][:, c, :].bitcast(F32),
                                            in1=PB[:], op=OP.mult)
                    nc.vector.tensor_reduce(FEATC[:, c, ex:ex + 1], scrF[:],
                                            axis=AX.X, op=OP.add)
                for c in range(HC):
                    nc.vector.tensor_copy(FEATC[:, HC + c, ex:ex + 1],
                                          SEQT[ex][:, c, 0:1].bitcast(F32))
            WANS0 = wts.tile([128, 2 * HC, H], BF16, tag="wbig", bufs=1, name="WANS0")
            nc.sync.dma_start(WANS0[:], wans0_d[:].rearrange("(c p) j -> p c j", p=128))
            FB = sm.tile([128, 2 * HC, PEREX], BF16)
            nc.vector.tensor_copy(FB[:], FEATC[:])
            psa = [ps.tile([PEREX, 512], F32, tag="psm", bufs=1, name=f"psa{i}") for i in range(2)]
            for nchunk in range(2):
                for c in range(2 * HC):
                    nc.tensor.matmul(psa[nchunk][:], lhsT=FB[:, c, :],
                                     rhs=WANS0[:, c, nchunk * 512:(nchunk + 1) * 512],
                                     start=(c == 0), stop=(c == 2 * HC - 1))
            ANS = sm.tile([PEREX, H], F32, tag="b4k", bufs=3, name="ANS")
            nc.vector.tensor_tensor(out=ANS[:, 0:512], in0=psa[0][:],
                                    in1=BANS0[:, 0:512], op=OP.add)
            nc.vector.tensor_tensor(out=ANS[:, 512:1024], in0=psa[1][:],
                                    in1=BANS0[:, 512:1024], op=OP.add)
            TANS = sm.tile([PEREX, H], F32, tag="b4k", bufs=3, name="ANS")
            nc.scalar.activation(TANS[:], ANS[:], AF.Tanh)
            scrA = sm.tile([PEREX, H], F32, tag="b4k", bufs=3, name="scrA")
            CLS = sm.tile([PEREX, 1], F32)
            nc.vector.tensor_tensor(out=scrA[:], in0=TANS[:], in1=WANS1[:], op=OP.mult)
            nc.vector.tensor_reduce(CLS[:], scrA[:], axis=AX.X, op=OP.add)
            nc.sync.dma_start(cls_d[:], CLS[:])

    nc.compile()
    return nc


def _host_prep(inputs):
    seq = np.asarray(inputs["sequence_output"], np.float32)
    pm = np.asarray(inputs["p_mask"], np.float32)
    w_start = np.asarray(inputs["w_start"], np.float32)[:, 0]
    b_start = float(np.asarray(inputs["b_start"], np.float32)[0])
    w_end0 = np.asarray(inputs["w_end0"], np.float32)
    b_end0 = np.asarray(inputs["b_end0"], np.float32)
    ln_g = np.asarray(inputs["ln_g"], np.float32)
    ln_b = np.asarray(inputs["ln_b"], np.float32)
    w_end1 = np.asarray(inputs["w_end1"], np.float32)[:, 0]
    b_end1 = float(np.asarray(inputs["b_end1"], np.float32)[0])
    w_ans0 = np.asarray(inputs["w_ans0"], np.float32)
    b_ans0 = np.asarray(inputs["b_ans0"], np.float32)
    w_ans1 = np.asarray(inputs["w_ans1"], np.float32)[:, 0]

    import ml_dtypes
    gw = (np.float64(ln_g) * np.float64(w_end1)).astype(np.float32)
    c_gw = float(np.float64(gw).sum())
    c_bw = float((np.float64(ln_b) * np.float64(w_end1)).sum() + b_end1)

    # pka: [128, 32] = wst(8) | onesgw(16, interleaved [c][2]) | b0c(8)
    pka = np.zeros((128, 32), np.float32)
    pka[:, 0:8] = w_start.reshape(HC, 128).T
    og = np.stack([np.ones((128, HC), np.float32), gw.reshape(HC, 128).T],
                  axis=2)                      # [128, HC, 2]
    pka[:, 8:24] = og.reshape(128, 16)
    pka[:, 24:32] = b_end0.reshape(HC, 128).T

    consts = np.zeros((16, 8), np.float32)
    consts[:, C_GW] = c_gw
    consts[:, C_BW] = c_bw
    consts[:, C_BST] = b_start
    consts[:, C_EPS] = 1e-12
    consts[:, C_NGW] = -c_gw
    cio = np.zeros(48, np.float32)
    for q in range(48):
        cio[q] = 512.0 * ((q // 3) % 8)
    iota = (float(S - 1) - np.arange(S, dtype=np.float32))

    common = {
        "w0a": np.ascontiguousarray(w_end0[:H]),
        "w0b": np.ascontiguousarray(w_end0[H:]),
        "wans0": w_ans0.astype(ml_dtypes.bfloat16),
        "pka": pka,
        "pkc": np.concatenate([np.broadcast_to(w_ans1, (PEREX, H)),
                               np.broadcast_to(b_ans0, (PEREX, H))],
                              axis=1).astype(np.float32),
    }

    in_maps = []
    for core in range(NCORES):
        sl = slice(core * PEREX, (core + 1) * PEREX)
        keep = (1.0 - pm[sl]).astype(np.float32)
        negm = (-1e30 * pm[sl]).astype(np.float32)
        pkb = np.zeros((16, 1608), np.float32)
        pkb[0:NPAIR, 0:S] = np.repeat(keep, K1, axis=0)
        pkb[0:NPAIR, S:2 * S] = np.repeat(negm, K1, axis=0)
        pkb[0:NPAIR, 2 * S:3 * S] = iota[None, :]
        pkb[0:16, 3 * S:3 * S + 8] = consts
        pkb[0:1, 3 * S + 8:3 * S + 56] = cio[None, :]
        m = dict(common)
        m["seqT"] = np.ascontiguousarray(seq[sl].transpose(0, 2, 1))
        m["pkb"] = pkb
        in_maps.append(m)
    return in_maps


def kernel(**inputs):
    assert int(inputs["start_n_top"]) == K1 and int(inputs["end_n_top"]) == K2
    if "nc" not in _CACHED:
        _CACHED["nc"] = build_nc()
    nc = _CACHED["nc"]
    in_maps = _host_prep(inputs)
    trace = os.environ.get("KERNEL_TRACE", "") == "1"
    res = run_bass_kernel_spmd(nc, in_maps, core_ids=list(range(NCORES)),
                               trace=trace)
    _CACHED["last_result"] = res
    rs = res.results
    slp = np.concatenate([r["slp"] for r in rs], axis=0)
    sidx = np.concatenate([r["sidx"] for r in rs], axis=0).astype(np.int32)
    elp = np.concatenate([r["elp"] for r in rs], axis=0)
    eidx = np.concatenate([r["eidx"] for r in rs], axis=0).astype(np.int32)
    cls = np.concatenate([r["cls"] for r in rs], axis=0)[:, 0]
    return slp, sidx, elp, eidx, cls
